# revision 21
# baseline (speedup 1.0000x reference)
"""Trainium2 Bass kernel for nn_LinearWithGroupedConv (out = x @ weight.T).

Full-input contract: kernel(x=[4,2048,4096] f32, weight=[4096,4096] f32)
-> [4,2048,4096] f32.

Strategy (tensor-parallel, column sharding per the hint):
  - out[s, o] = sum_k x[s, k] * weight[o, k];  S=8192 (4*2048), K=4096, O=4096.
  - Shard `weight` over out_feature across 8 cores (512 columns each),
    replicate x. Each core computes out_shard [8192, 512]; host concats.
  - fp16 operands (PSUM accumulation is fp32; rel err ~2e-4).
  - Host pre-lays-out x as [128p, chunk, ktile, s] and w as [128p, ktile, o]
    so every DMA moves large per-partition-contiguous lines (a [K, S]
    layout produces 1 KB descriptors that throttle the rings to ~half rate
    and starve the PE at chunk boundaries).
  - Head: ten N=512 warm-up matmuls on a memset tile run during the first
    DMA wait so the PE HAM clock-gate reaches 8/8 (2.4 GHz) before real
    work; the first 1024 columns are processed as one k-interleaved
    "superchunk" across all 8 PSUM banks, with w and x streamed JIT in
    fine k-blocks balanced across both HWDGE rings -- per k-step the PE
    does 8 matmuls (~1.7us) against ~384 KB of streamed data, matching
    ring delivery so the cold start has no long stalls.
  - Steady state: 256-column chunks, single 2 MB contiguous DMA each,
    alternating rings with 2-chunk lookahead; outputs ride the HWDGE ring
    opposite the chunk's input ring.  Measured: every matmul issues at the
    215.7 ns NX floor (LDWEIGHTS fully hidden, zero PE gaps).
  - Per chunk: 2 PSUM accumulation groups x 32 k-tile matmuls
    ([128,128] stationary x [128,512] moving), DVE copy to SBUF, DMA out;
    the last chunk runs ss-outer so its first store overlaps its matmuls.
"""

import numpy as np

import concourse.bass as bass
import concourse.mybir as mybir
import concourse.tile as tile
from concourse import bacc
from concourse.bass_utils import run_bass_kernel_spmd

N_CORES = 8
S = 8192          # 4 * 2048 sequence rows
K = 4096          # in_feature (contraction)
O = 4096          # out_feature
O_SHARD = O // N_CORES          # 512
P = 128
K_TILES = K // P                # 32
S_CHUNK = 256                   # seq columns per streamed x chunk
S_SUB = S_CHUNK // P            # 2 psum groups per chunk
N_CHUNKS = S // S_CHUNK         # 32
N_SUPER = 4                     # chunks fused into the k-interleaved head
# Fine-grained early blocks: the PE consumes k-tiles faster than the rings
# can deliver during the cold start, so small gates keep each stall tiny
# (well under the ~3.4us HAM re-throttle window).
K_BLOCKS = [(0, 1), (1, 2), (2, 4), (4, 8), (8, 12), (12, 16),
            (16, 20), (20, 24), (24, 28), (28, 32)]

MODE = "fp16"            # informational; single fp16 path
PROFILE = False          # test.py sets True to capture an NTFF trace
LAST_PROFILE = None      # BassKernelResults of the last run when PROFILE

_CACHE = {}


def _build(dt16=mybir.dt.float16):
    nc = bacc.Bacc(None, target_bir_lowering=False)

    x = nc.dram_tensor("x", [P, N_CHUNKS, K_TILES, S_CHUNK], dt16,
                       kind="ExternalInput")
    w = nc.dram_tensor("w", [P, K_TILES, O_SHARD], dt16, kind="ExternalInput")
    out = nc.dram_tensor("out", [S, O_SHARD], mybir.dt.float32,
                         kind="ExternalOutput")

    with tile.TileContext(nc) as tc:
        with (
            tc.tile_pool(name="wpool", bufs=1) as wpool,
            tc.tile_pool(name="x0pool", bufs=1) as x0pool,
            tc.tile_pool(name="xpool", bufs=2) as xpool,
            tc.tile_pool(name="opool", bufs=4) as opool,
            tc.tile_pool(name="warmsb", bufs=1) as warmsb,
            tc.tile_pool(name="psum", bufs=8, space=bass.MemorySpace.PSUM) as psum,
        ):
            # PE warm-up during the DMA head: HAM un-throttles after ~3.4us
            # of sustained PE activity, so a burst of tiny matmuls here gets
            # the clock to 2.4 GHz sooner than the first real matmul would.
            # The warm PSUM tile shares the "pt" tag (slot rotation frees it
            # long before the slot is reused).
            # N=512 bursts at the cold clock take ~0.53us each; ten of them
            # span the ~5.5us wait for the first k-tile to land, so HAM
            # reaches 8/8 before the first real matmul instead of 6us in.
            warm_sb = warmsb.tile([P, O_SHARD], dt16)
            nc.gpsimd.memset(warm_sb[:], 0.0)
            warm_ps = psum.tile([P, O_SHARD], mybir.dt.float32, tag="pt",
                                name="warm")
            for _ in range(10):
                nc.tensor.matmul(warm_ps[:], warm_sb[:, 0:P], warm_sb[:],
                                 start=True, stop=True)

            # Superchunk head streaming: one w DMA + one x DMA per k-block,
            # the two pieces of each block on OPPOSITE rings (parallel
            # arrival), alternating per block for byte balance.  Fewer head
            # DMAs matters: completion-sem lanes are recycled round-robin
            # and a ring stalls ~2us whenever it reuses a lane whose
            # previous DMA hasn't confirmed receipt yet.
            w_sb = []
            x0_sb = []
            for bi, (k0, k1) in enumerate(K_BLOCKS):
                weng, xeng = ((nc.sync, nc.scalar) if bi % 2 == 0
                              else (nc.scalar, nc.sync))
                wt = wpool.tile([P, k1 - k0, O_SHARD], dt16, name=f"w_{k0}")
                weng.dma_start(wt[:], w[:, k0:k1, :])
                w_sb.append(wt)
                xt = x0pool.tile([P, N_SUPER, k1 - k0, S_CHUNK], dt16,
                                 name=f"x0_{k0}")
                xeng.dma_start(xt[:], x[:, 0:N_SUPER, k0:k1, :])
                x0_sb.append(xt)

            # Later chunks: single 2 MB fully-contiguous DMA each, alternating
            # rings.  The first two are emitted here; the rest are emitted
            # inside the chunk loop (2-chunk lookahead) so their pool-slot
            # waits resolve just before the data is needed.
            xc_sb = {}

            def emit_xc(c):
                t = xpool.tile([P, K_TILES, S_CHUNK], dt16, tag="xc", name="xc")
                eng = nc.scalar if (c % 2 == 1) else nc.sync
                eng.dma_start(t[:], x[:, c, :, :])
                xc_sb[c] = t

            for c in range(N_SUPER, min(N_SUPER + 2, N_CHUNKS)):
                emit_xc(c)

            def w_ap(k):
                for bi, (k0, k1) in enumerate(K_BLOCKS):
                    if k0 <= k < k1:
                        return w_sb[bi][:, k - k0, :]

            # Superchunk 0: the first N_SUPER*S_CHUNK columns processed
            # k-interleaved across N_SUPER*S_SUB PSUM groups.  Per k-step the
            # PE does 8 matmuls (~1.7us warm) against 384KB of JIT-streamed
            # w+x -- matching the ring delivery rate, so the head has no
            # long PE stalls and HAM warms once.
            n_grp = N_SUPER * S_SUB
            pts = [
                psum.tile([P, O_SHARD], mybir.dt.float32, tag="pt",
                          name=f"pt{g}")
                for g in range(n_grp)
            ]
            def super_mm(k, g):
                for bi, (k0, k1) in enumerate(K_BLOCKS):
                    if k0 <= k < k1:
                        kk = k - k0
                        break
                c, s2 = divmod(g, S_SUB)
                nc.tensor.matmul(
                    pts[g][:],
                    x0_sb[bi][:, c, kk, s2 * P:(s2 + 1) * P],
                    w_ap(k),
                    start=(k == 0),
                    stop=(k == K_TILES - 1),
                )

            # k-major while streaming; the tail (last 8 k) runs GROUP-major
            # so group 0's accumulation stops ~12us before the superchunk
            # ends -- its PSUM bank is copied out and free before the first
            # steady chunk asks for it (else that chunk's first matmuls
            # stall ~2.4us on the copy semaphore).
            K_TAIL = 24
            for k in range(K_TAIL):
                for g in range(n_grp):
                    super_mm(k, g)
            for g in range(n_grp):
                for k in range(K_TAIL, K_TILES):
                    super_mm(k, g)
            for g in range(n_grp):
                o_sb = opool.tile([P, O_SHARD], mybir.dt.float32)
                nc.vector.tensor_copy(o_sb[:], pts[g][:])
                oeng = nc.sync if (g % 2 == 1) else nc.scalar
                oeng.dma_start(out[g * P:(g + 1) * P, :], o_sb[:])

            # Steady-state chunks.
            for c in range(N_SUPER, N_CHUNKS):
                if c + 2 < N_CHUNKS and c + 2 >= N_SUPER + 2:
                    emit_xc(c + 2)
                pts = [
                    psum.tile([P, O_SHARD], mybir.dt.float32, tag="pt",
                              name=f"pt{ss}")
                    for ss in range(S_SUB)
                ]
                # Outputs ride the HWDGE ring opposite to this chunk's input
                # ring (SWDGE drains ~6us at kernel end; HWDGE doesn't).
                oeng = nc.sync if (c % 2 == 1) else nc.scalar

                def flush(ss):
                    o_sb = opool.tile([P, O_SHARD], mybir.dt.float32)
                    nc.vector.tensor_copy(o_sb[:], pts[ss][:])
                    s0 = c * S_CHUNK + ss * P
                    oeng.dma_start(out[s0:s0 + P, :], o_sb[:])

                if c < N_CHUNKS - 1:
                    for k in range(K_TILES):
                        for ss in range(S_SUB):
                            nc.tensor.matmul(
                                pts[ss][:],
                                xc_sb[c][:, k, ss * P:(ss + 1) * P],
                                w_ap(k),
                                start=(k == 0),
                                stop=(k == K_TILES - 1),
                            )
                    for ss in range(S_SUB):
                        flush(ss)
                else:
                    # Last chunk runs ss-outer so group 0's copy+store
                    # overlaps group 1's matmuls, shortening the kernel tail.
                    for ss in range(S_SUB):
                        for k in range(K_TILES):
                            nc.tensor.matmul(
                                pts[ss][:],
                                xc_sb[c][:, k, ss * P:(ss + 1) * P],
                                w_ap(k),
                                start=(k == 0),
                                stop=(k == K_TILES - 1),
                            )
                        flush(ss)
    nc.compile()
    return nc


def _install_ntff_hook():
    """Register the axon NTFF profiling hook if the image's antenv lacks it.

    Only used when PROFILE=True (test harness); grading never hits this.
    """
    import sys
    import types

    if "antenv.axon_hooks" in sys.modules:
        return
    try:
        from trn_agent_boot.trn_boot import _ntff_profile_via_ctypes
    except ImportError:
        return
    try:
        hook = _ntff_profile_via_ctypes("/opt/axon/libaxon_pjrt.so")
    except OSError:
        return
    m = types.ModuleType("antenv.axon_hooks")
    m.get_axon_ntff_profile_hook = lambda: hook
    m.set_axon_ntff_profile_hook = lambda h: None
    sys.modules["antenv.axon_hooks"] = m


def _get_nc():
    if "nc" not in _CACHE:
        _CACHE["nc"] = _build()
    return _CACHE["nc"]


def kernel(x: np.ndarray, weight: np.ndarray) -> np.ndarray:
    global LAST_PROFILE
    b, s, k = x.shape
    assert (b * s, k) == (S, K) and weight.shape == (O, K)

    xT16 = np.ascontiguousarray(x.reshape(S, K).T).astype(np.float16)  # [K, S]
    # [ktile, p, chunk, s] -> [p, chunk, ktile, s]: every chunk slice is
    # per-partition contiguous (16 KB lines).
    x_arr = np.ascontiguousarray(
        xT16.reshape(K_TILES, P, N_CHUNKS, S_CHUNK).transpose(1, 2, 0, 3))
    wT16 = weight.T.astype(np.float16)                                 # [K, O]

    in_maps = []
    for c in range(N_CORES):
        w_c = np.ascontiguousarray(
            wT16[:, c * O_SHARD:(c + 1) * O_SHARD]
            .reshape(K_TILES, P, O_SHARD).transpose(1, 0, 2))
        in_maps.append({"x": x_arr, "w": w_c})

    if PROFILE:
        _install_ntff_hook()
    nc = _get_nc()
    res = run_bass_kernel_spmd(
        nc,
        in_maps,
        core_ids=list(range(N_CORES)),
        trace=PROFILE,
        trace_cores=[0] if PROFILE else None,
    )
    LAST_PROFILE = res

    full = np.empty((S, O), dtype=np.float32)
    for c in range(N_CORES):
        full[:, c * O_SHARD:(c + 1) * O_SHARD] = res.results[c]["out"]
    return full.reshape(b, s, O)
